# revision 28
# baseline (speedup 1.0000x reference)
"""Trainium2 Bass kernel for nn_BinaryQuantumClassifier.

Math: the 4-qubit circuit collapses to a closed form. Per sample, with
theta_j = pi * (x @ W_ctq.T + b_ctq)_j  (j = 4r + i, reuse r, qubit i):
    d_i(theta) = a_i + b_i sin(theta) + c_i cos(theta)
              = a_i + R_i sin(pi * (y + b_ctq_j + phi_i/pi))
(R = hypot(b, c), phi = atan2(c, b); a/b/c derived from the fixed per-qubit
unitary RZ RY RX after RY(theta) H|0>), and the CNOT chain maps
Z-expectations to products of the d_i:
    z0 = d1 d2 d3, z1 = d0 d1, z2 = d0 d1 d2, z3 = d0 d1 d2 d3.
Output = (mean over r of z) @ W_cls.T + b_cls.
The R factors are divided out of d (d' = s + a/R) and folded into the
final linear weights, so the epilogue per value is one add after the sin.

Device plan per core (8192 samples). The kernel is HBM-bound on reading x,
so x is sent as fp16 (2 B/elem, ~8.4 MB/core; fp16's 10 mantissa bits keep
the final rel err ~2.5e-3, well under the gate). x is relayouted on the
host so the PE uses it as the STATIONARY operand (FWL fast weight load),
W as the tiny moving operand:
  lhsT = x-chunk [128 D x 128 samples], rhs = W-chunk [128 D x 8] fp16,
  out[128 samples, 8] accumulated over 4 D-chunks in fp32 PSUM.
x arrives as 8 x 1 MB DMAs, all issued up-front on the
sync ring (one sequential queue => data streams continuously at the HBM
roofline); the first two are hoisted ahead of the framework entry barrier.
Constants ride the ACT ring.
Epilogue per QUARTER (16 groups, [128, 128]-wide tiles, so DVE ops are
wide and few — each DVE op costs ~150-200 ns of fixed overhead; fp16
intermediates get the DVE 2x perf modes):
  E = psum + phase-shift (fp32, free = j*16 + u), k2 = ((E + 1.5*2^24) -
  1.5*2^24) rounds to the nearest even integer (exact range reduction;
  the HW Sin table is garbage outside ~one period), r = E - k2 in [-1, 1]
  (fp16), ScalarE Sin, d' = s + a/R, CNOT products as 3 wide muls on
  strided views (PQT slots z3|z2|z0|z1|e, so the four z products end up
  contiguous), then r-mean + 4->2 linear FULLY FOLDED into 5 wide ops:
  P_c = z-slots * CW_c, two s-tree adds, one r-pair add, one bias add;
  one [128, 32] fp16 output tile per quarter, DMA'd on the ACT ring as
  soon as it is ready. For the LAST quarter the pre-sin stages and the
  products run per 8-group half, pipelined with the tail of the x
  stream, so only a half-width pre-chain plus the 5-op tree remain after
  the final x byte lands.
"""

import numpy as np

import concourse.bass as bass
import concourse.mybir as mybir
from concourse import bass_utils
from concourse.tile import TileContext

B, D, NQ = 65536, 512, 4
NCORES = 8
BC = B // NCORES            # 8192 samples per core
NCH = D // 128              # 4 K-chunks
FE = BC // 128              # 64 sample-groups per core (epilogue u index)
M2 = float(np.float32(1.5 * 2 ** 24))   # round-to-even-integer magic
PI = float(np.pi)
MM_DT = mybir.dt.float16    # PE operand dtype (x and W both fp16)
EP_DT = mybir.dt.float16    # epilogue intermediate dtype (DVE 2x/4x modes)
NG = BC // 128              # 64 sample-groups per core
NQT = 4                     # epilogue quarters
FQ = 16                     # groups per quarter
JQ = 8 * FQ                 # 128: width of a quarter's epilogue tile (j*16+u)
TGRP = [8, 8, 8, 8, 8, 8, 8, 8]         # groups per x DMA tile
AL = mybir.AluOpType
AF = mybir.ActivationFunctionType
F32 = mybir.dt.float32


def _split_waits(nc, max_waits=1):
    """walrus in this env accepts at most one sync-wait per instruction;
    move extras onto preceding same-engine NoOps."""
    for fn in nc.m.functions:
        for blk in fn.blocks:
            new_list = []
            for inst in blk.instructions:
                si = inst.sync_info
                if si is not None and len(si.on_wait) > max_waits:
                    waits = list(si.on_wait)
                    keep, extra = waits[-max_waits:], waits[:-max_waits]
                    for k, w in enumerate(extra):
                        new_list.append(mybir.InstNoOp(
                            name=f"{inst.name}-ws{k}", engine=inst.engine,
                            ins=[], outs=[],
                            sync_info=mybir.SyncInfo(on_wait=[w], on_update=[])))
                    si.on_wait = keep
                    inst.sync_info = si
                new_list.append(inst)
            blk.instructions = new_list


def _hoist_dmas(nc, n_sync=2, n_act=2):
    """Move the first wait-free DMA issues of the sync (x tiles) and ACT
    (constants) rings from the tile block into `main`, ahead of the
    all-engine entry barrier, so the x stream starts ~1 us earlier.
    Best-effort: on any unexpected module shape, leave the program as-is
    (still correct, just ~1 us slower)."""
    try:
        _hoist_dmas_inner(nc, n_sync, n_act)
    except Exception:
        pass


def _hoist_dmas_inner(nc, n_sync, n_act):
    blks = [b for f in nc.m.functions for b in f.blocks]
    main = next(b for b in blks if b.name == "main")
    tile = max(blks, key=lambda b: len(b.instructions))
    want = {mybir.EngineType.SP: n_sync, mybir.EngineType.Activation: n_act}
    hoisted, rest = [], []
    for inst in tile.instructions:
        if (want.get(inst.engine, 0) > 0 and isinstance(inst, mybir.InstDMACopy)
                and not (inst.sync_info and inst.sync_info.on_wait)):
            hoisted.append(inst)
            want[inst.engine] -= 1
        else:
            rest.append(inst)
    tile.instructions = rest
    # insert at the very start of main (before register moves/barrier);
    # the DMA APs use static addresses, not the R8/R10-13 queue regs
    main.instructions = hoisted + main.instructions


def _build_nc():
    nc = bass.Bass("TRN2", target_bir_lowering=False)
    # x relayout (fp16): xa[p, m*512 + k*128 + ms] = x[128m + ms, 128k + p]
    xa_d = nc.dram_tensor("xa", [128, BC * NCH], MM_DT, kind="ExternalInput").ap()
    # W chunks: [k*8 + j] = W.T chunk k (fp16); partition row 0 also holds
    # a ones row [32:160] and the phase-shift rows bshi [160:168],
    # bslo [168:176] — added into PSUM via two K=1 matmuls per group
    wcat_d = nc.dram_tensor("wcat", [128, 176], MM_DT, kind="ExternalInput").ap()
    # fp16 consts: CW [128, 256] | CB [128, 32]
    cf16_d = nc.dram_tensor("cf16", [128, 9 * 2 * FQ],
                            EP_DT, kind="ExternalInput").ap()
    # o[p, 32*qi + 16*c + uq] = out_c(sample 128*(16*qi + uq) + p), fp16
    o_d = nc.dram_tensor("o", [128, 2 * FE], EP_DT, kind="ExternalOutput").ap()

    tile_g0 = np.cumsum([0] + TGRP)     # first group of each x tile

    with TileContext(nc) as tc:
        with tc.tile_pool(name="wp", bufs=1) as wpool, \
             tc.tile_pool(name="xp", bufs=len(TGRP)) as xpool, \
             tc.tile_pool(name="pp", bufs=4, space="PSUM") as pspool, \
             tc.tile_pool(name="ep", bufs=1) as epool:
            # constants on the ACT ring (parallel with x on the sync ring)
            wsb = wpool.tile([128, 176], MM_DT)
            nc.scalar.dma_start(wsb[:], wcat_d[:])
            c16 = wpool.tile([128, 9 * 2 * FQ], EP_DT)
            nc.scalar.dma_start(c16[:], cf16_d[:])
            ones = wsb[0:1, 32:160]
            bshi = wsb[0:1, 160:168]
            bslo = wsb[0:1, 168:176]
            CW = c16[:, 0:8 * 2 * FQ]
            CB = c16[:, 8 * 2 * FQ:9 * 2 * FQ]

            # all x DMAs up-front, one sequential queue
            Oall = epool.tile([128, 2 * FE], EP_DT, name="Oall")
            Las = []
            for t, gw in enumerate(TGRP):
                gl = gw * NCH * 128
                La = xpool.tile([128, gl], MM_DT, tag="La", name=f"La{t}")
                off = int(tile_g0[t]) * NCH * 128
                nc.sync.dma_start(La[:], xa_d[:, off:off + gl])
                Las.append(La)

            for qi in range(NQT):
                # last quarter: split the pre-sin stages per 8-group half so
                # only half-width work remains after the final x byte lands
                nh = 2 if qi == NQT - 1 else 1
                psqs = [pspool.tile([128, JQ // nh], F32, tag=f"ps{h}",
                                    name=f"ps{qi}_{h}") for h in range(nh)]
                for t, gw in enumerate(TGRP):
                    for lg in range(gw):
                        g = int(tile_g0[t]) + lg
                        if not (FQ * qi <= g < FQ * (qi + 1)):
                            continue
                        mq = g - FQ * qi
                        h = mq * nh // FQ
                        mh = mq - h * (FQ // nh)
                        out_sl = psqs[h][:, 8 * mh:8 * mh + 8]
                        for k in range(NCH):
                            off = lg * (NCH * 128) + k * 128
                            nc.tensor.matmul(out_sl, Las[t][:, off:off + 128],
                                             wsb[:, 8 * k:8 * k + 8],
                                             start=(k == 0), stop=False)
                        nc.tensor.matmul(out_sl, ones, bshi,
                                         start=False, stop=False)
                        nc.tensor.matmul(out_sl, ones, bslo,
                                         start=False, stop=True)

                # ---- epilogue for this quarter (j*16 + u layout) ----
                # psum already holds E = y + bs; the qubit DC term a is
                # structurally 0 (the state sweeps a great circle), so
                # d = R sin(pi r) with R folded into CW — no adds needed
                k2 = epool.tile([128, JQ], F32, name=f"k2_{qi}")
                r_ = epool.tile([128, JQ], EP_DT, name=f"r_{qi}")
                s_ = epool.tile([128, JQ], EP_DT, name=f"s_{qi}")
                FH = FQ // nh
                for h in range(nh):
                    psr = psqs[h].rearrange("p (m j) -> p j m", j=8)
                    k2v = k2.rearrange("p (j u) -> p j u", j=8)[:, :, FH * h:FH * (h + 1)]
                    rv = r_.rearrange("p (j u) -> p j u", j=8)[:, :, FH * h:FH * (h + 1)]
                    sv = s_.rearrange("p (j u) -> p j u", j=8)[:, :, FH * h:FH * (h + 1)]
                    nc.vector.tensor_scalar(k2v, psr, M2, M2, AL.add, AL.subtract)
                    nc.vector.tensor_sub(rv, psr, k2v)     # E mod 2 -> [-1, 1]
                    nc.scalar.activation(sv, rv, AF.Sin, scale=PI)

                # CNOT products, 3 wide muls on strided views (per half
                # for the last quarter so they pipeline with the x stream).
                # d4[i, r, u] = d'(j = 4r + i); PQT slots s: z3, z2, z0, z1, e
                d4 = s_.rearrange("p (r i u) -> p i r u", r=2, i=4)
                PQT = epool.tile([128, 5 * 2 * FQ], EP_DT, name=f"PQT{qi}")
                P5 = PQT.rearrange("p (s r u) -> p s r u", s=5, r=2)
                for h in range(nh):
                    d4h = d4[:, :, :, FH * h:FH * (h + 1)]
                    sl = (slice(None), slice(None), slice(None),
                          slice(FH * h, FH * (h + 1)))
                    nc.vector.tensor_mul(P5[:, 3:5][sl], d4h[:, 0:2],
                                         d4h[:, 1:3])                     # z1, e
                    nc.vector.tensor_mul(P5[:, 1:3][sl], P5[:, 3:5][sl],
                                         d4h[:, 2:4])                     # z2, z0
                    nc.vector.tensor_mul(P5[:, 0:1][sl], P5[:, 1:2][sl],
                                         d4h[:, 3:4])                     # z3
                # r-mean + final 4->2 linear, fully folded:
                # out_c = b_c + sum_{s,r} w''_{c,s} PQT[s, r, u]
                # (w'' = 0.5 * R-prod * W_cls, slots s0..3 = z3 z2 z0 z1)
                Pz4 = PQT.rearrange("p (c s v) -> p c s v", c=1, s=5)[:, :, 0:4]
                Pm = epool.tile([128, 2 * 4 * 2 * FQ], EP_DT, name=f"Pm{qi}")
                Pt = Pm.rearrange("p (c s v) -> p c s v", c=2, s=4)
                cw4 = CW.rearrange("p (c s v) -> p c s v", c=2, s=4)
                nc.vector.tensor_mul(Pt[:, 0:1], Pz4, cw4[:, 0:1])
                nc.vector.tensor_mul(Pt[:, 1:2], Pz4, cw4[:, 1:2])
                T1 = epool.tile([128, 2 * 2 * 2 * FQ], EP_DT, name=f"T1{qi}")
                t1v = T1.rearrange("p (c s v) -> p c s v", c=2, s=2)
                nc.vector.tensor_add(t1v[:], Pt[:, :, 0:2], Pt[:, :, 2:4])
                T2 = epool.tile([128, 2 * 2 * FQ], EP_DT, name=f"T2{qi}")
                t2v = T2.rearrange("p (c s v) -> p c s v", c=2, s=1)
                nc.vector.tensor_add(t2v[:], t1v[:, :, 0:1], t1v[:, :, 1:2])
                # T2[c, r*16+u]: sum the r halves, add bias
                t2r = T2.rearrange("p (c r u) -> p c r u", c=2, r=2)
                T3 = epool.tile([128, 2 * FQ], EP_DT, name=f"T3{qi}")
                t3v = T3.rearrange("p (c r u) -> p c r u", c=2, r=1)
                nc.vector.tensor_add(t3v[:], t2r[:, :, 0:1], t2r[:, :, 1:2])
                nc.vector.tensor_add(Oall[:, 2 * FQ * qi:2 * FQ * (qi + 1)],
                                     T3[:], CB)
            nc.scalar.dma_start(o_d[:], Oall[:])

    return nc


_NC_CACHE = {}


def _get_nc(split=True):
    key = ("nc", split)
    if key not in _NC_CACHE:
        nc = _build_nc()
        _hoist_dmas(nc)
        if split:
            _split_waits(nc)
        _NC_CACHE[key] = nc
    return _NC_CACHE[key]


def _qubit_abc(q_params):
    """Exact (a_i, b_i, c_i) with d_i(theta) = a + b sin(theta) + c cos(theta)."""
    out = np.zeros((NQ, 3), np.float64)
    for i in range(NQ):
        pa, pb, pc = [float(v) for v in q_params[3 * i:3 * i + 3]]

        def rx(t):
            return np.array([[np.cos(t / 2), -1j * np.sin(t / 2)],
                             [-1j * np.sin(t / 2), np.cos(t / 2)]])

        def ry(t):
            return np.array([[np.cos(t / 2), -np.sin(t / 2)],
                             [np.sin(t / 2), np.cos(t / 2)]])

        def rz(t):
            return np.array([[np.exp(-0.5j * t), 0], [0, np.exp(0.5j * t)]])

        H = np.array([[1, 1], [1, -1]]) / np.sqrt(2)
        U = rz(pc) @ ry(pb) @ rx(pa)

        def dfun(theta):
            v = U @ ry(theta) @ H @ np.array([1.0, 0.0])
            pr = np.abs(v) ** 2
            return pr[0] - pr[1]

        d0, dpi, dh = dfun(0.0), dfun(np.pi), dfun(np.pi / 2)
        a = (d0 + dpi) / 2
        c = (d0 - dpi) / 2
        b = dh - a
        out[i] = (a, b, c)
    return out


def _make_consts(b_ctq, q_params, W_cls, b_cls):
    abc = _qubit_abc(q_params)
    Rq = np.maximum(np.hypot(abc[:, 1], abc[:, 2]), 1e-3)       # [4]
    gam = np.array([Rq[1] * Rq[2] * Rq[3], Rq[0] * Rq[1],
                    Rq[0] * Rq[1] * Rq[2], Rq[0] * Rq[1] * Rq[2] * Rq[3]])
    bs = np.array([b_ctq[j] + np.arctan2(abc[j % 4][2], abc[j % 4][1]) / np.pi
                   for j in range(8)], np.float32)
    cf16 = np.zeros((128, 9 * 2 * FQ), np.float16)
    wp = 0.5 * np.asarray(W_cls, np.float64) * gam[None, :]     # mean + R folded
    SLOTK = (3, 2, 0, 1)            # PQT slot s -> z_k index
    V = 2 * FQ
    for c in range(2):
        for s in range(4):
            cf16[:, (4 * c + s) * V:(4 * c + s + 1) * V] = \
                np.float16(wp[c, SLOTK[s]])
    cf16[:, 8 * V:8 * V + FQ] = np.float16(b_cls[0])
    cf16[:, 8 * V + FQ:9 * V] = np.float16(b_cls[1])
    return bs, cf16


def make_in_maps(x, W_ctq, b_ctq, q_params, W_cls, b_cls):
    f16 = np.float16
    wt = np.asarray(W_ctq, np.float32).T                        # [512, 8]
    bs, cf16 = _make_consts(np.asarray(b_ctq, np.float32),
                            np.asarray(q_params, np.float32),
                            np.asarray(W_cls, np.float32),
                            np.asarray(b_cls, np.float32))
    wcat = np.zeros((128, 176), f16)
    for k in range(NCH):
        wcat[:, 8 * k:8 * (k + 1)] = wt[128 * k:128 * (k + 1), :].astype(f16)
    wcat[0, 32:160] = f16(1.0)                  # ones row for the K=1 matmuls
    bshi = bs.astype(f16)
    bslo = (bs - bshi.astype(np.float32)).astype(f16)
    wcat[0, 160:168] = bshi
    wcat[0, 168:176] = bslo
    x = np.asarray(x, np.float32)
    in_maps = []
    for c in range(NCORES):
        xs = x[c * BC:(c + 1) * BC]                             # [8192, 512]
        # relayout: [p, m*512 + k*128 + ms] = xs[128 m + ms, 128 k + p]
        xa = np.ascontiguousarray(
            xs.reshape(NG, 128, NCH, 128).transpose(3, 0, 2, 1)
            .reshape(128, BC * NCH).astype(f16))
        in_maps.append({"xa": xa, "wcat": wcat, "cf16": cf16})
    return in_maps


def assemble_output(results):
    out = np.empty((B, 2), np.float32)
    for core in range(NCORES):
        o = np.asarray(results[core]["o"], np.float32)           # [128, 2*FE]
        # o[p, 32*qi + 16*c + uq] = out_c(sample 128*(16*qi + uq) + p)
        out[core * BC:(core + 1) * BC] = (
            o.reshape(128, NQT, 2, FQ).transpose(1, 3, 0, 2).reshape(BC, 2))
    return out


def kernel(x, W_ctq, b_ctq, q_params, W_cls, b_cls):
    nc = _get_nc()
    in_maps = make_in_maps(x, W_ctq, b_ctq, q_params, W_cls, b_cls)
    res = bass_utils.run_bass_kernel_spmd(nc, in_maps, core_ids=list(range(NCORES)))
    return assemble_output(res.results)


# revision 29
# speedup vs baseline: 1.0769x; 1.0769x over previous
"""Trainium2 Bass kernel for nn_BinaryQuantumClassifier.

Math: the 4-qubit circuit collapses to a closed form. Per sample, with
theta_j = pi * (x @ W_ctq.T + b_ctq)_j  (j = 4r + i, reuse r, qubit i):
    d_i(theta) = a_i + b_i sin(theta) + c_i cos(theta)
              = a_i + R_i sin(pi * (y + b_ctq_j + phi_i/pi))
(R = hypot(b, c), phi = atan2(c, b); a/b/c derived from the fixed per-qubit
unitary RZ RY RX after RY(theta) H|0>), and the CNOT chain maps
Z-expectations to products of the d_i:
    z0 = d1 d2 d3, z1 = d0 d1, z2 = d0 d1 d2, z3 = d0 d1 d2 d3.
Output = (mean over r of z) @ W_cls.T + b_cls.
The R factors are divided out of d (d' = s + a/R) and folded into the
final linear weights, so the epilogue per value is one add after the sin.

Device plan per core (8192 samples). The kernel is HBM-bound on reading x,
so x is sent as fp16 (2 B/elem, ~8.4 MB/core; fp16's 10 mantissa bits keep
the final rel err ~2.5e-3, well under the gate). x is relayouted on the
host so the PE uses it as the STATIONARY operand (FWL fast weight load),
W as the tiny moving operand:
  lhsT = x-chunk [128 D x 128 samples], rhs = W-chunk [128 D x 8] fp16,
  out[128 samples, 8] accumulated over 4 D-chunks in fp32 PSUM.
x arrives as 8 x 1 MB DMAs, all issued up-front on the
sync ring (one sequential queue => data streams continuously at the HBM
roofline); the first two are hoisted ahead of the framework entry barrier.
Constants ride the ACT ring.
Epilogue per QUARTER (16 groups, [128, 128]-wide tiles, so DVE ops are
wide and few — each DVE op costs ~150-200 ns of fixed overhead; fp16
intermediates get the DVE 2x perf modes):
  E = psum + phase-shift (fp32, free = j*16 + u), k2 = ((E + 1.5*2^24) -
  1.5*2^24) rounds to the nearest even integer (exact range reduction;
  the HW Sin table is garbage outside ~one period), r = E - k2 in [-1, 1]
  (fp16), ScalarE Sin, d' = s + a/R, CNOT products as 3 wide muls on
  strided views (PQT slots z3|z2|z0|z1|e, so the four z products end up
  contiguous), then r-mean + 4->2 linear FULLY FOLDED into 5 wide ops:
  P_c = z-slots * CW_c, two s-tree adds, one r-pair add, one bias add;
  one [128, 32] fp16 output tile per quarter, DMA'd on the ACT ring as
  soon as it is ready. For the LAST quarter the pre-sin stages and the
  products run per 8-group half, pipelined with the tail of the x
  stream, so only a half-width pre-chain plus the 5-op tree remain after
  the final x byte lands.
"""

import numpy as np

import concourse.bass as bass
import concourse.mybir as mybir
from concourse import bass_utils
from concourse.tile import TileContext

B, D, NQ = 65536, 512, 4
NCORES = 8
BC = B // NCORES            # 8192 samples per core
NCH = D // 128              # 4 K-chunks
FE = BC // 128              # 64 sample-groups per core (epilogue u index)
M2 = float(np.float32(1.5 * 2 ** 24))   # round-to-even-integer magic
PI = float(np.pi)
MM_DT = mybir.dt.float16    # PE operand dtype (x and W both fp16)
EP_DT = mybir.dt.float16    # epilogue intermediate dtype (DVE 2x/4x modes)
NG = BC // 128              # 64 sample-groups per core
NQT = 4                     # epilogue quarters
FQ = 16                     # groups per quarter
JQ = 8 * FQ                 # 128: width of a quarter's epilogue tile (j*16+u)
TGRP = [8, 8, 8, 8, 8, 8, 8, 8]         # groups per x DMA tile
AL = mybir.AluOpType
AF = mybir.ActivationFunctionType
F32 = mybir.dt.float32


def _split_waits(nc, max_waits=1):
    """walrus in this env accepts at most one sync-wait per instruction;
    move extras onto preceding same-engine NoOps."""
    for fn in nc.m.functions:
        for blk in fn.blocks:
            new_list = []
            for inst in blk.instructions:
                si = inst.sync_info
                if si is not None and len(si.on_wait) > max_waits:
                    waits = list(si.on_wait)
                    keep, extra = waits[-max_waits:], waits[:-max_waits]
                    for k, w in enumerate(extra):
                        new_list.append(mybir.InstNoOp(
                            name=f"{inst.name}-ws{k}", engine=inst.engine,
                            ins=[], outs=[],
                            sync_info=mybir.SyncInfo(on_wait=[w], on_update=[])))
                    si.on_wait = keep
                    inst.sync_info = si
                new_list.append(inst)
            blk.instructions = new_list


def _hoist_dmas(nc, n_sync=2, n_act=3):
    """Move the first wait-free DMA issues of the sync (x tiles) and ACT
    (constants) rings from the tile block into `main`, ahead of the
    all-engine entry barrier, so the x stream starts ~1 us earlier.
    Best-effort: on any unexpected module shape, leave the program as-is
    (still correct, just ~1 us slower)."""
    try:
        _hoist_dmas_inner(nc, n_sync, n_act)
    except Exception:
        pass


def _hoist_dmas_inner(nc, n_sync, n_act):
    blks = [b for f in nc.m.functions for b in f.blocks]
    main = next(b for b in blks if b.name == "main")
    tile = max(blks, key=lambda b: len(b.instructions))
    want = {mybir.EngineType.SP: n_sync, mybir.EngineType.Activation: n_act}
    hoisted, rest = [], []
    for inst in tile.instructions:
        if (want.get(inst.engine, 0) > 0 and isinstance(inst, mybir.InstDMACopy)
                and not (inst.sync_info and inst.sync_info.on_wait)):
            hoisted.append(inst)
            want[inst.engine] -= 1
        else:
            rest.append(inst)
    tile.instructions = rest
    # insert at the very start of main (before register moves/barrier);
    # the DMA APs use static addresses, not the R8/R10-13 queue regs
    main.instructions = hoisted + main.instructions


def _build_nc():
    nc = bass.Bass("TRN2", target_bir_lowering=False)
    # x relayout (fp16): xa[p, m*512 + k*128 + ms] = x[128m + ms, 128k + p]
    xa_d = nc.dram_tensor("xa", [128, BC * NCH], MM_DT, kind="ExternalInput").ap()
    # W chunks: [k*8 + j] = W.T chunk k (fp16)
    wcat_d = nc.dram_tensor("wcat", [128, 32], MM_DT, kind="ExternalInput").ap()
    # fp32 consts: bsT [128, JQ] (phase shift, j*16+u)
    cf32_d = nc.dram_tensor("cf32", [128, JQ], F32, kind="ExternalInput").ap()
    # fp16 consts: AoR [128, JQ] | CW [128, 256] | CB [128, 32]
    cf16_d = nc.dram_tensor("cf16", [128, JQ + 9 * 2 * FQ],
                            EP_DT, kind="ExternalInput").ap()
    # o[p, 32*qi + 16*c + uq] = out_c(sample 128*(16*qi + uq) + p), fp16
    o_d = nc.dram_tensor("o", [128, 2 * FE], EP_DT, kind="ExternalOutput").ap()

    tile_g0 = np.cumsum([0] + TGRP)     # first group of each x tile

    with TileContext(nc) as tc:
        with tc.tile_pool(name="wp", bufs=1) as wpool, \
             tc.tile_pool(name="xp", bufs=len(TGRP)) as xpool, \
             tc.tile_pool(name="pp", bufs=4, space="PSUM") as pspool, \
             tc.tile_pool(name="ep", bufs=1) as epool:
            # constants on the ACT ring (parallel with x on the sync ring)
            wsb = wpool.tile([128, 32], MM_DT)
            nc.scalar.dma_start(wsb[:], wcat_d[:])
            c32 = wpool.tile([128, JQ], F32)
            nc.scalar.dma_start(c32[:], cf32_d[:])
            c16 = wpool.tile([128, JQ + 9 * 2 * FQ], EP_DT)
            nc.scalar.dma_start(c16[:], cf16_d[:])
            bs3 = c32.rearrange("p (j u) -> p j u", j=8)
            AoR = c16[:, 0:JQ]
            CW = c16[:, JQ:JQ + 8 * 2 * FQ]
            CB = c16[:, JQ + 8 * 2 * FQ:JQ + 9 * 2 * FQ]

            # all x DMAs up-front, one sequential queue
            Las = []
            for t, gw in enumerate(TGRP):
                gl = gw * NCH * 128
                La = xpool.tile([128, gl], MM_DT, tag="La", name=f"La{t}")
                off = int(tile_g0[t]) * NCH * 128
                nc.sync.dma_start(La[:], xa_d[:, off:off + gl])
                Las.append(La)

            for qi in range(NQT):
                # last quarter: split the pre-sin stages per 8-group half so
                # only half-width work remains after the final x byte lands
                nh = 2 if qi == NQT - 1 else 1
                psqs = [pspool.tile([128, JQ // nh], F32, tag=f"ps{h}",
                                    name=f"ps{qi}_{h}") for h in range(nh)]
                for t, gw in enumerate(TGRP):
                    for lg in range(gw):
                        g = int(tile_g0[t]) + lg
                        if not (FQ * qi <= g < FQ * (qi + 1)):
                            continue
                        mq = g - FQ * qi
                        h = mq * nh // FQ
                        mh = mq - h * (FQ // nh)
                        for k in range(NCH):
                            off = lg * (NCH * 128) + k * 128
                            nc.tensor.matmul(psqs[h][:, 8 * mh:8 * mh + 8],
                                             Las[t][:, off:off + 128],
                                             wsb[:, 8 * k:8 * k + 8],
                                             start=(k == 0), stop=(k == NCH - 1))

                # ---- epilogue for this quarter (j*16 + u layout) ----
                E = epool.tile([128, JQ], F32, name=f"E{qi}")
                k2 = epool.tile([128, JQ], F32, name=f"k2_{qi}")
                r_ = epool.tile([128, JQ], EP_DT, name=f"r_{qi}")
                s_ = epool.tile([128, JQ], EP_DT, name=f"s_{qi}")
                d_ = epool.tile([128, JQ], EP_DT, name=f"d_{qi}")
                FH = FQ // nh
                for h in range(nh):
                    # E[:, 16j + m] = ps[:, 8m + j] + bs (phase shift)
                    e3 = E.rearrange("p (j u) -> p j u", j=8)[:, :, FH * h:FH * (h + 1)]
                    bsh = bs3[:, :, FH * h:FH * (h + 1)]
                    nc.vector.tensor_add(e3, psqs[h].rearrange("p (m j) -> p j m", j=8),
                                         bsh)
                    ev = E.rearrange("p (j u) -> p j u", j=8)[:, :, FH * h:FH * (h + 1)]
                    k2v = k2.rearrange("p (j u) -> p j u", j=8)[:, :, FH * h:FH * (h + 1)]
                    rv = r_.rearrange("p (j u) -> p j u", j=8)[:, :, FH * h:FH * (h + 1)]
                    sv = s_.rearrange("p (j u) -> p j u", j=8)[:, :, FH * h:FH * (h + 1)]
                    dv = d_.rearrange("p (j u) -> p j u", j=8)[:, :, FH * h:FH * (h + 1)]
                    av = AoR.rearrange("p (j u) -> p j u", j=8)[:, :, FH * h:FH * (h + 1)]
                    nc.vector.tensor_scalar(k2v, ev, M2, M2, AL.add, AL.subtract)
                    nc.vector.tensor_sub(rv, ev, k2v)      # E mod 2 -> [-1, 1]
                    nc.scalar.activation(sv, rv, AF.Sin, scale=PI)
                    nc.vector.tensor_add(dv, sv, av)       # d' = sin + a/R

                # CNOT products, 3 wide muls on strided views (per half
                # for the last quarter so they pipeline with the x stream).
                # d4[i, r, u] = d'(j = 4r + i); PQT slots s: z3, z2, z0, z1, e
                d4 = d_.rearrange("p (r i u) -> p i r u", r=2, i=4)
                PQT = epool.tile([128, 5 * 2 * FQ], EP_DT, name=f"PQT{qi}")
                P5 = PQT.rearrange("p (s r u) -> p s r u", s=5, r=2)
                for h in range(nh):
                    d4h = d4[:, :, :, FH * h:FH * (h + 1)]
                    sl = (slice(None), slice(None), slice(None),
                          slice(FH * h, FH * (h + 1)))
                    nc.vector.tensor_mul(P5[:, 3:5][sl], d4h[:, 0:2],
                                         d4h[:, 1:3])                     # z1, e
                    nc.vector.tensor_mul(P5[:, 1:3][sl], P5[:, 3:5][sl],
                                         d4h[:, 2:4])                     # z2, z0
                    nc.vector.tensor_mul(P5[:, 0:1][sl], P5[:, 1:2][sl],
                                         d4h[:, 3:4])                     # z3
                # r-mean + final 4->2 linear, fully folded:
                # out_c = b_c + sum_{s,r} w''_{c,s} PQT[s, r, u]
                # (w'' = 0.5 * R-prod * W_cls, slots s0..3 = z3 z2 z0 z1)
                Pz4 = PQT.rearrange("p (c s v) -> p c s v", c=1, s=5)[:, :, 0:4]
                Pm = epool.tile([128, 2 * 4 * 2 * FQ], EP_DT, name=f"Pm{qi}")
                Pt = Pm.rearrange("p (c s v) -> p c s v", c=2, s=4)
                cw4 = CW.rearrange("p (c s v) -> p c s v", c=2, s=4)
                nc.vector.tensor_mul(Pt[:, 0:1], Pz4, cw4[:, 0:1])
                nc.vector.tensor_mul(Pt[:, 1:2], Pz4, cw4[:, 1:2])
                T1 = epool.tile([128, 2 * 2 * 2 * FQ], EP_DT, name=f"T1{qi}")
                t1v = T1.rearrange("p (c s v) -> p c s v", c=2, s=2)
                nc.vector.tensor_add(t1v[:], Pt[:, :, 0:2], Pt[:, :, 2:4])
                T2 = epool.tile([128, 2 * 2 * FQ], EP_DT, name=f"T2{qi}")
                t2v = T2.rearrange("p (c s v) -> p c s v", c=2, s=1)
                nc.vector.tensor_add(t2v[:], t1v[:, :, 0:1], t1v[:, :, 1:2])
                # T2[c, r*16+u]: sum the r halves, add bias
                t2r = T2.rearrange("p (c r u) -> p c r u", c=2, r=2)
                T3 = epool.tile([128, 2 * FQ], EP_DT, name=f"T3{qi}")
                t3v = T3.rearrange("p (c r u) -> p c r u", c=2, r=1)
                nc.vector.tensor_add(t3v[:], t2r[:, :, 0:1], t2r[:, :, 1:2])
                Oq = epool.tile([128, 2 * FQ], EP_DT, name=f"Oq{qi}")
                nc.vector.tensor_add(Oq[:], T3[:], CB)
                nc.scalar.dma_start(o_d[:, 2 * FQ * qi:2 * FQ * (qi + 1)], Oq[:])

    return nc


_NC_CACHE = {}


def _get_nc(split=True):
    key = ("nc", split)
    if key not in _NC_CACHE:
        nc = _build_nc()
        _hoist_dmas(nc)
        if split:
            _split_waits(nc)
        _NC_CACHE[key] = nc
    return _NC_CACHE[key]


def _qubit_abc(q_params):
    """Exact (a_i, b_i, c_i) with d_i(theta) = a + b sin(theta) + c cos(theta)."""
    out = np.zeros((NQ, 3), np.float64)
    for i in range(NQ):
        pa, pb, pc = [float(v) for v in q_params[3 * i:3 * i + 3]]

        def rx(t):
            return np.array([[np.cos(t / 2), -1j * np.sin(t / 2)],
                             [-1j * np.sin(t / 2), np.cos(t / 2)]])

        def ry(t):
            return np.array([[np.cos(t / 2), -np.sin(t / 2)],
                             [np.sin(t / 2), np.cos(t / 2)]])

        def rz(t):
            return np.array([[np.exp(-0.5j * t), 0], [0, np.exp(0.5j * t)]])

        H = np.array([[1, 1], [1, -1]]) / np.sqrt(2)
        U = rz(pc) @ ry(pb) @ rx(pa)

        def dfun(theta):
            v = U @ ry(theta) @ H @ np.array([1.0, 0.0])
            pr = np.abs(v) ** 2
            return pr[0] - pr[1]

        d0, dpi, dh = dfun(0.0), dfun(np.pi), dfun(np.pi / 2)
        a = (d0 + dpi) / 2
        c = (d0 - dpi) / 2
        b = dh - a
        out[i] = (a, b, c)
    return out


def _make_consts(b_ctq, q_params, W_cls, b_cls):
    abc = _qubit_abc(q_params)
    Rq = np.maximum(np.hypot(abc[:, 1], abc[:, 2]), 1e-3)       # [4]
    gam = np.array([Rq[1] * Rq[2] * Rq[3], Rq[0] * Rq[1],
                    Rq[0] * Rq[1] * Rq[2], Rq[0] * Rq[1] * Rq[2] * Rq[3]])
    cf32 = np.zeros((128, JQ), np.float32)
    cf16 = np.zeros((128, JQ + 9 * 2 * FQ), np.float16)
    for j in range(8):
        i = j % 4
        a, b, c_ = abc[i]
        phi = np.arctan2(c_, b)
        cf32[:, j * FQ:(j + 1) * FQ] = np.float32(b_ctq[j] + phi / np.pi)
        cf16[:, j * FQ:(j + 1) * FQ] = np.float16(a / Rq[i])
    wp = 0.5 * np.asarray(W_cls, np.float64) * gam[None, :]     # mean + R folded
    SLOTK = (3, 2, 0, 1)            # PQT slot s -> z_k index
    V = 2 * FQ
    for c in range(2):
        for s in range(4):
            cf16[:, JQ + (4 * c + s) * V:JQ + (4 * c + s + 1) * V] = \
                np.float16(wp[c, SLOTK[s]])
    cf16[:, JQ + 8 * V:JQ + 8 * V + FQ] = np.float16(b_cls[0])
    cf16[:, JQ + 8 * V + FQ:JQ + 9 * V] = np.float16(b_cls[1])
    return cf32, cf16


def make_in_maps(x, W_ctq, b_ctq, q_params, W_cls, b_cls):
    f16 = np.float16
    wt = np.asarray(W_ctq, np.float32).T                        # [512, 8]
    wcat = np.zeros((128, 32), f16)
    for k in range(NCH):
        wcat[:, 8 * k:8 * (k + 1)] = wt[128 * k:128 * (k + 1), :].astype(f16)
    cf32, cf16 = _make_consts(np.asarray(b_ctq, np.float32),
                              np.asarray(q_params, np.float32),
                              np.asarray(W_cls, np.float32),
                              np.asarray(b_cls, np.float32))
    x = np.asarray(x, np.float32)
    in_maps = []
    for c in range(NCORES):
        xs = x[c * BC:(c + 1) * BC]                             # [8192, 512]
        # relayout: [p, m*512 + k*128 + ms] = xs[128 m + ms, 128 k + p]
        xa = np.ascontiguousarray(
            xs.reshape(NG, 128, NCH, 128).transpose(3, 0, 2, 1)
            .reshape(128, BC * NCH).astype(f16))
        in_maps.append({"xa": xa, "wcat": wcat, "cf32": cf32, "cf16": cf16})
    return in_maps


def assemble_output(results):
    out = np.empty((B, 2), np.float32)
    for core in range(NCORES):
        o = np.asarray(results[core]["o"], np.float32)           # [128, 2*FE]
        # o[p, 32*qi + 16*c + uq] = out_c(sample 128*(16*qi + uq) + p)
        out[core * BC:(core + 1) * BC] = (
            o.reshape(128, NQT, 2, FQ).transpose(1, 3, 0, 2).reshape(BC, 2))
    return out


def kernel(x, W_ctq, b_ctq, q_params, W_cls, b_cls):
    nc = _get_nc()
    in_maps = make_in_maps(x, W_ctq, b_ctq, q_params, W_cls, b_cls)
    res = bass_utils.run_bass_kernel_spmd(nc, in_maps, core_ids=list(range(NCORES)))
    return assemble_output(res.results)
